# revision 1
# baseline (speedup 1.0000x reference)
"""Trainium2 Bass kernel for nn_Attention_45148696216373.

8-core data-parallel over tokens (B*S = 131072 -> 16384/core); x is
pre-transposed on the host to [128c, tokens] so channel sits on SBUF
partitions for the PE matmul. All scalar constants (1/sqrt(D), the
1/H of the head-mean) and the output projection Wo are folded into a
single 960-wide fused projection [Q(256)|msum(64)|K0(128)|VW(512)]
run at full PE rate via float32r (1 cyc/row at N>=256). Bias rides as
a K=1 ones-row matmul accumulated into the same PSUM banks. The
per-token bilinear part (attention scores + output combine) runs on
the Vector engine in fp16 at 2x perf mode, staged PSUM->SBUF by the
Scalar engine; per-op overhead amortized by spanning 8 token-tiles
per instruction. Modeled: ~271us/core, DVE-bound (90%), vs ~71us DMA
roofline; rel err ~7e-4.

Math (per token t, all ops independent across tokens):
  q_st = x @ Wq^T + bq   -> [D,H] raw-reshaped to [H,D]  (index scramble)
  k_st, v_st similarly -> [KV,D]
  msum = sum_h q[h,:]                      (mean*4; /4 folded into Wvo)
  km   = k0 * msum                         (k scaled by q-mean)
  attn[h,k] = sum_d q[h,d]*km[k,d]         (/sqrt(D) folded into Wvo)
  out[h,:]  = sum_k attn[h,k]*v[k,:]
  y = reshape(out)[2 rows of 128] @ Wo^T

Device formulation: fold Wo into the V projection on the host:
  VW[t,(r,k),o] = (1/32) * sum_d v[t,k*64+d] * Wo[o, r*64+d]
  Y[t, j*128+o] = sum_{r,k} attn_raw[t,2j+r,k] * VW[t,(2r+k)*128+o]
One 960-wide fused projection per token: [Q(256) | msum(64) | K0(128) | VW(512)].
"""

import os

# The Bass SPMD path needs the axon trn2 PJRT backend; a cpu pin (e.g. from a
# reference-only harness env) would hide the 8 NeuronCores from jax.devices().
if os.environ.get("JAX_PLATFORMS", "").strip().lower() == "cpu":
    os.environ.pop("JAX_PLATFORMS")

import numpy as np

B, S, DIM = 16, 8192, 128
H, KV, D = 4, 2, 64
T = B * S                 # 131072 tokens
NCORES = 8
TPC = T // NCORES         # 16384 tokens per core
TT = 128                  # tokens per tile (partition dim)
NT = TPC // TT            # 128 tiles per core

NQ = H * D                # 256
NM = D                    # 64
NK = KV * D               # 128
NV = 4 * DIM              # 512 (VW block: q=(2r+k) blocks of 128)
NPROJ = NQ + NM + NK + NV  # 960
OQ, OM, OK, OV = 0, NQ, NQ + NM, NQ + NM + NK

_COMPILED = None


def _fold_weights(Wq, bq, Wk, bk, Wv, bv, Wo):
    """Build W_all [128, 960] and bias_all [960] (fp32)."""
    j = np.arange(NQ)
    Wq_p = Wq[j % H, j // H, :]            # [256, 128]
    bq_p = bq[j % H, j // H]               # [256]
    jk = np.arange(NK)
    Wk_p = Wk[jk % KV, jk // KV, :]        # [128, 128]
    bk_p = bk[jk % KV, jk // KV]
    Wv_p = Wv[jk % KV, jk // KV, :]        # [128, 128]
    bv_p = bv[jk % KV, jk // KV]

    # msum block: col d = sum_h Wq_p[h*64+d]
    Wm = Wq_p.reshape(H, D, DIM).sum(axis=0)     # [64, 128]
    bm = bq_p.reshape(H, D).sum(axis=0)          # [64]

    # VW block: row (q=2r+k)*128+o = (1/32) sum_d Wv_p[k*64+d,:]*Wo[o, r*64+d]
    Wvo = np.zeros((4, DIM, DIM), dtype=np.float64)
    bvo = np.zeros((4, DIM), dtype=np.float64)
    scale = 1.0 / 32.0
    for r in range(2):
        for k in range(2):
            q = 2 * r + k
            # [o, c] = sum_d Wo[o, r*64+d] * Wv_p[k*64+d, c]
            Wvo[q] = scale * (Wo[:, r * D:(r + 1) * D] @ Wv_p[k * D:(k + 1) * D, :])
            bvo[q] = scale * (Wo[:, r * D:(r + 1) * D] @ bv_p[k * D:(k + 1) * D])

    # VW block stored [o-major, q-minor]: col = OV + o*4 + q  (enables
    # innermost-q step-1 access in the Y-combine for DVE 2x mode)
    Wvo_oq = Wvo.transpose(1, 0, 2).reshape(4 * DIM, DIM)
    bvo_oq = bvo.T.reshape(4 * DIM)
    W_all = np.concatenate(
        [Wq_p, Wm, Wk_p, Wvo_oq], axis=0
    ).astype(np.float32)                               # [960, 128]
    b_all = np.concatenate(
        [bq_p, bm, bk_p, bvo_oq]
    ).astype(np.float32)                               # [960]
    return W_all.T.copy(), b_all                       # [128, 960], [960]


def _numpy_forward(x2d, W_all, b_all):
    """Host re-implementation of the device math (for validation)."""
    proj = x2d @ W_all + b_all                         # [t, 960]
    Q = proj[:, OQ:OQ + NQ].reshape(-1, H, D)
    msum = proj[:, OM:OM + NM]
    K0 = proj[:, OK:OK + NK].reshape(-1, KV, D)
    VW = proj[:, OV:OV + NV].reshape(-1, DIM, 4)   # [t, o, q]
    km = K0 * msum[:, None, :]
    attn = np.einsum("thd,tkd->thk", Q, km)            # [t, 4, 2]
    a = attn.reshape(-1, 2, 4)                         # [t, j, q=(2r+k)]
    # Y[t, j, o] = sum_q a[t,j,q] * VW[t, o, q]
    Y = np.einsum("tjq,toq->tjo", a, VW)
    return Y.reshape(-1, 2 * DIM)                      # [t, 256]


def _build_program():
    import concourse.bass as bass
    import concourse.tile as tile
    from concourse import bacc, mybir

    f32 = mybir.dt.float32
    f32r = mybir.dt.float32r
    bf16 = mybir.dt.float16  # fp16: same 2x DVE modes as bf16, 4x less rounding error

    nc = bacc.Bacc(
        "TRN2",
        target_bir_lowering=False,
        debug=False,
        enable_asserts=False,
        num_devices=NCORES,
    )

    xT_d = nc.dram_tensor("xT", [DIM, TPC], f32r, kind="ExternalInput").ap()
    w_d = nc.dram_tensor("wall", [DIM, NPROJ], f32r, kind="ExternalInput").ap()
    b_d = nc.dram_tensor("ball", [1, NPROJ], f32r, kind="ExternalInput").ap()
    one_d = nc.dram_tensor("ones", [1, TT], f32r, kind="ExternalInput").ap()
    y_d = nc.dram_tensor("y", [TPC, 2 * DIM], f32, kind="ExternalOutput").ap()

    with tile.TileContext(nc) as tc:
        with (
            tc.tile_pool(name="const", bufs=1) as cpool,
            tc.tile_pool(name="xin", bufs=3) as xpool,
            tc.tile_pool(name="psum", bufs=2, space="PSUM") as ppool,
            tc.tile_pool(name="work", bufs=3) as wpool,
            tc.tile_pool(name="yout", bufs=2) as ypool,
        ):
            w_sb = cpool.tile([DIM, NPROJ], f32r)
            nc.sync.dma_start(w_sb[:], w_d[:, :])
            b_sb = cpool.tile([1, NPROJ], f32r)
            nc.sync.dma_start(b_sb[:], b_d[:, :])
            one_sb = cpool.tile([1, TT], f32r)
            nc.sync.dma_start(one_sb[:], one_d[:, :])

            G = 2   # PSUM group (2 x [TT,2,1024]f32 tiles double-buffered = 16KB)
            GS = 8  # SBUF group: DVE/ACT ops span 8 tiles to amortize op overhead
            for g in range(NT // GS):
                xt = xpool.tile([DIM, GS * TT], f32r)
                nc.sync.dma_start(xt[:], xT_d[:, g * GS * TT:(g + 1) * GS * TT])

                stg = wpool.tile([TT, GS, 960], bf16)
                for sub in range(GS // G):
                    pp = ppool.tile([TT, G, 1024], f32, name=f"pp{sub}", tag="pp")
                    for v in range(G):
                        u = sub * G + v
                        for lo, hi in ((0, 512), (512, NPROJ)):
                            nc.tensor.matmul(
                                out=pp[:, v, lo:hi],
                                lhsT=one_sb[:, :],
                                rhs=b_sb[:, lo:hi],
                                start=True, stop=False,
                            )
                            nc.tensor.matmul(
                                out=pp[:, v, lo:hi],
                                lhsT=xt[:, u * TT:(u + 1) * TT],
                                rhs=w_sb[:, lo:hi],
                                start=False, stop=True,
                            )
                    # Stage this PSUM pair -> its half of the fp16 SBUF group
                    nc.scalar.copy(
                        stg[:, sub * G:(sub + 1) * G, :], pp[:, :, 0:960]
                    )

                qmk = stg
                vw = stg[:, :, OV:OV + NV].rearrange("p g (o q) -> p g o q", q=4)

                # km[g,k,d] = K0[g,k,d] * msum[g,d]   (fp16, 2x)
                km = wpool.tile([TT, GS, KV, D], bf16)
                nc.vector.tensor_mul(
                    km[:],
                    qmk[:, :, OK:OK + NK].rearrange("p g (k d) -> p g k d", k=KV),
                    qmk[:, :, OM:OM + NM].unsqueeze(2).broadcast_to([TT, GS, KV, D]),
                )

                # P[g,h,k,d] = Q[g,h,d] * km[g,k,d]   (fp16, 2x; per-u: 3 free dims max)
                P = wpool.tile([TT, GS, H, KV, D], bf16)
                for u in range(GS):
                    nc.vector.tensor_mul(
                        P[:, u],
                        qmk[:, u, OQ:OQ + NQ]
                        .rearrange("p (h d) -> p h d", h=H)
                        .unsqueeze(2)
                        .broadcast_to([TT, H, KV, D]),
                        km[:, u].unsqueeze(1).broadcast_to([TT, H, KV, D]),
                    )

                # attn[g,h,k] = sum_d P: fold tree (2x adds) then 1x reduce
                A1 = wpool.tile([TT, GS, 8, 32], bf16)
                Pf = P[:].rearrange("p g h k d -> p g (h k) d")
                nc.vector.tensor_add(A1[:], Pf[:, :, :, 0:32], Pf[:, :, :, 32:64])
                A2 = wpool.tile([TT, GS, 8, 16], bf16)
                nc.vector.tensor_add(A2[:], A1[:, :, :, 0:16], A1[:, :, :, 16:32])
                attn = wpool.tile([TT, GS, 8], f32)
                nc.vector.tensor_reduce(
                    attn[:], A2[:], axis=mybir.AxisListType.X,
                    op=mybir.AluOpType.add,
                )
                attnb = wpool.tile([TT, GS, 8], bf16)
                nc.vector.tensor_copy(attnb[:], attn[:])

                # YP[g,j,o,q] = attn[g,j,q] * VW[g,o,q]   (fp16, 2x; per-u)
                YP = wpool.tile([TT, GS, 2, DIM, 4], bf16)
                for u in range(GS):
                    nc.vector.tensor_mul(
                        YP[:, u],
                        attnb[:, u].rearrange("p (j q) -> p j q", j=2)
                        .unsqueeze(2).broadcast_to([TT, 2, DIM, 4]),
                        vw[:, u].unsqueeze(1).broadcast_to([TT, 2, DIM, 4]),
                    )

                # Y[g,j,o] = sum_q YP: pair-fold (2x) + final add (1x, fp32)
                # (g,j) merged into one AP dim to stay within 3 free dims
                F = wpool.tile([TT, GS, 2, DIM, 2], bf16)
                YPm = YP[:].rearrange("p g j o q -> p (g j) o q")
                nc.vector.tensor_add(
                    F[:].rearrange("p g j o q -> p (g j) o q"),
                    YPm[:, :, :, 0:2], YPm[:, :, :, 2:4],
                )
                Y = ypool.tile([TT, GS, 2 * DIM], f32)
                Fm = F[:].rearrange("p g j o q -> p (g j) o q")
                nc.vector.tensor_add(
                    Y[:].rearrange("p g (j o) -> p (g j) o", j=2),
                    Fm[:, :, :, 0], Fm[:, :, :, 1],
                )

                for u in range(GS):
                    nc.sync.dma_start(
                        y_d[(g * GS + u) * TT:(g * GS + u + 1) * TT, :],
                        Y[:, u, :],
                    )

    nc.compile()
    return nc


def kernel(x, Wq, bq, Wk, bk, Wv, bv, Wo):
    global _COMPILED
    from concourse.bass_utils import run_bass_kernel_spmd

    x = np.asarray(x, dtype=np.float32)
    W_all, b_all = _fold_weights(
        np.asarray(Wq, np.float32), np.asarray(bq, np.float32),
        np.asarray(Wk, np.float32), np.asarray(bk, np.float32),
        np.asarray(Wv, np.float32), np.asarray(bv, np.float32),
        np.asarray(Wo, np.float32),
    )

    if _COMPILED is None:
        _COMPILED = _build_program()
    nc = _COMPILED

    x2d = x.reshape(T, DIM)
    ones = np.ones((1, TT), dtype=np.float32)
    ball = b_all.reshape(1, NPROJ)
    in_maps = []
    for c in range(NCORES):
        shard = x2d[c * TPC:(c + 1) * TPC]          # [16384, 128]
        in_maps.append({
            "xT": np.ascontiguousarray(shard.T),     # [128, 16384]
            "wall": W_all,
            "ball": ball,
            "ones": ones,
        })

    res = run_bass_kernel_spmd(nc, in_maps, list(range(NCORES)))
    ys = [res.results[c]["y"] for c in range(NCORES)]
    Y = np.concatenate(ys, axis=0)                   # [131072, 256]
    return Y.reshape(B, 2 * S, DIM)

